# revision 56
# baseline (speedup 1.0000x reference)
"""Distributed Bass kernel for nn_Attention (B=2, S=2048, HID=2048, H=32, KVH=8, D=64).

Sharding (8 NeuronCores, uniform SPMD graph):
  - Head-parallel phase: core c owns kv-head c and its 4 GQA query heads.
    x replicated as xT [HID, B*S] bf16; per-core Q^T [256, 4096] (+RoPE,
    1/sqrt(D) folded into trig tables), K^T [64, 4096] (+RoPE, duplicated to
    rows 64:128), V [keys, 64|1] blocks with a ones-column for the softmax
    denominator.
  - Attention in S^T layout [keys, queries], processed per head-PAIR: the two
    heads of a pair sit on PE row-groups 0/64, so their K=64 S^T matmuls run
    concurrently (row-group tiling).  exp is split between the Scalar engine
    (true Exp) and the Vector engine (one-instruction Schraudolph bf16
    bit-trick exp ~ bitcast(int16(x*128/ln2 + 16248.75))) so neither engine
    bottlenecks.  Causal masking: rectangular blocks need no mask; the 4
    diagonal-band blocks per chunk use width-trimmed matmuls plus a 0/1 mask
    multiply (split Vector/GpSimd).
  - Per-unit drains DMA straight into the AllToAll staging buffer
    [NC, 130, TL] whose rows 64/129 carry the (bf16) softmax denominators —
    one collective per head-pair, no separate denominator collective.  Pair 0's
    collective overlaps pair 1's attention.
  - Token-parallel output projection: ao tiles normalized post-collective,
    out rows = ao.T @ wo per 512-col block; wo prefetched during attention.
"""

import os
import numpy as np
import ml_dtypes

import concourse.bass as bass
import concourse.mybir as mybir
import concourse.tile as tile
from concourse import bacc
from concourse.bass_utils import run_bass_kernel_spmd

BF16 = ml_dtypes.bfloat16
F32 = np.float32

B, S, HID = 2, 2048, 2048
H, KVH, D = 32, 8, 64
NC = 8                 # cores
T = B * S              # 4096 flat tokens
TL = T // NC           # 512 tokens per core (phase-2 output rows)
LH = H // NC           # 4 local q-heads per core
KB = 128               # key block
TC = 512               # phase-1 token streaming chunk / query chunk
NTC = T // TC          # 8 token chunks

SCHR_A = float(128.0 / np.log(2.0))
SCHR_B = float(127.0 * 128.0 - 7.25)

_CACHE = {}


def _build():
    fp32 = mybir.dt.float32
    bf16 = mybir.dt.bfloat16
    i16 = mybir.dt.int16

    # knobs (read at build time; defaults are the tuned config)
    DVE_EXP = float(os.environ.get("KDVE", "0.5"))   # fraction of exps on DVE
    DIAG_TRIM = os.environ.get("KDIAG", "1") == "1"  # width-trim diagonal blocks

    fp8 = mybir.dt.float8e4
    DR = mybir.MatmulPerfMode.DoubleRow

    nc = bacc.Bacc("TRN2", target_bir_lowering=False, debug=False, num_devices=NC)

    xT = nc.dram_tensor("xT", [NTC, 128, 16, TC], bf16, kind="ExternalInput")
    wq_c = nc.dram_tensor("wq_c", [128, 16, LH * D], bf16, kind="ExternalInput")
    wkv_c = nc.dram_tensor("wkv_c", [128, 16, 2 * D], bf16, kind="ExternalInput")
    wo = nc.dram_tensor("wo", [4, 128, 16, 512], bf16, kind="ExternalInput")
    ctq_d = nc.dram_tensor("ctq", [128, 2, T], fp32, kind="ExternalInput")
    ctk_d = nc.dram_tensor("ctk", [64, 2, T], fp32, kind="ExternalInput")
    mq_d = nc.dram_tensor("mq", [128, 2, 128], bf16, kind="ExternalInput")
    out_d = nc.dram_tensor("out", [TL, HID], fp32, kind="ExternalOutput")

    with tile.TileContext(nc) as tc:
        with (
            tc.tile_pool(name="persist", bufs=1) as persist,
            tc.tile_pool(name="stream", bufs=2) as stream,
            tc.tile_pool(name="trig", bufs=2) as trig,
            tc.tile_pool(name="work", bufs=2) as work,
            tc.tile_pool(name="psum", bufs=1, space="PSUM") as psum,
            tc.tile_pool(name="dram", bufs=1, space="DRAM") as dram,
        ):
            # ---- persistent tiles ----
            qT = [persist.tile([128, T], bf16, tag=f"qT{t}", name=f"qT{t}")
                  for t in range(2)]
            k2 = persist.tile([128, T], bf16, tag="k2", name="k2")
            vatt = [persist.tile([128, D + 1], bf16, tag=f"vatt{i}", name=f"vatt{i}")
                    for i in range(T // KB)]
            msq = persist.tile([128, 2, 128], bf16, tag="msq", name="msq")
            ident = persist.tile([128, 128], bf16, tag="ident", name="ident")

            # ---- weights (merged single-DMA loads) ----
            wq_sb = persist.tile([128, 16, LH * D], bf16, tag="wq", name="wq")
            wkv_sb = persist.tile([128, 16, 2 * D], bf16, tag="wkv", name="wkv")
            for g in range(4):
                nc.scalar.dma_start(wq_sb[:, 4 * g:4 * (g + 1), :],
                                    wq_c[:, 4 * g:4 * (g + 1), :])
            nc.scalar.dma_start(wkv_sb[:], wkv_c[:])

            from concourse.masks import make_identity
            make_identity(nc, ident[:])

            # ================= Phase 1: QKV projections + RoPE =================
            def rope(out_ap, ps, ct, st, npart):
                """out = ps*ct + swap32(ps)*st  (st carries the rotate-half sign)."""
                t1 = work.tile([128, TC], fp32, tag="rope_t1", name="t1")
                t2 = work.tile([128, TC], fp32, tag="rope_t2", name="t2")
                nc.vector.tensor_mul(t1[:npart, :], ps[:npart, :], ct[:npart, :])
                for base in range(0, npart, 64):
                    a, b2 = base, base + 32
                    nc.vector.tensor_mul(t2[a:a + 32, :], ps[b2:b2 + 32, :], st[a:a + 32, :])
                    nc.vector.tensor_mul(t2[b2:b2 + 32, :], ps[a:a + 32, :], st[b2:b2 + 32, :])
                nc.vector.tensor_add(out_ap, t1[:npart, :], t2[:npart, :])

            for tc8 in range(NTC):
                tsl = slice(TC * tc8, TC * (tc8 + 1))
                xt = stream.tile([128, 16, TC], bf16, tag="s", name=f"x{tc8}")
                if tc8 == 0:
                    # split the first chunk across three queues so the pieces
                    # land in parallel and the first matmuls start early
                    for g, eng in enumerate((nc.sync, nc.scalar, nc.gpsimd,
                                             nc.sync)):
                        eng.dma_start(xt[:, 4 * g:4 * (g + 1), :],
                                      xT[tc8, :, 4 * g:4 * (g + 1), :])
                elif tc8 in (1, 2):
                    # gpsimd is idle at startup: prefetch chunks 1-2 there so
                    # they don't queue behind chunk 0's pieces
                    nc.gpsimd.dma_start(xt[:], xT[tc8])
                else:
                    nc.sync.dma_start(xt[:, 0:8, :], xT[tc8, :, 0:8, :])
                    nc.scalar.dma_start(xt[:, 8:16, :], xT[tc8, :, 8:16, :])
                ctq = trig.tile([128, 2, TC], fp32, tag="ctq", name="ctq")
                ctk = trig.tile([64, 2, TC], fp32, tag="ctk", name="ctk")
                # chunk 0's tables ride the idle gpsimd queue at startup so
                # they don't wait behind the weight loads on scalar
                teng = nc.gpsimd if tc8 == 0 else nc.scalar
                teng.dma_start(ctq[:], ctq_d[:, :, tsl])
                teng.dma_start(ctk[:], ctk_d[:, :, tsl])

                # Q^T: two 128-row tiles (2 heads each)
                for qt in range(2):
                    ps = psum.tile([128, 2 * TC], fp32, tag="big", bufs=3,
                                   name="ps_q")[:, 0:TC]
                    for k in range(16):
                        nc.tensor.matmul(ps[:], wq_sb[:, k, 128 * qt:128 * (qt + 1)],
                                         xt[:, k, :], start=(k == 0), stop=(k == 15))
                    rope(qT[qt][:, tsl], ps, ctq[:, 0, :], ctq[:, 1, :], 128)

                # K^T (rows 0:64) and V^T (rows 64:128) in one packed projection
                ps = psum.tile([128, 2 * TC], fp32, tag="big", bufs=3,
                               name="ps_kv")[:, 0:TC]
                for k in range(16):
                    nc.tensor.matmul(ps[:], wkv_sb[:, k, :], xt[:, k, :],
                                     start=(k == 0), stop=(k == 15))
                rope(k2[0:64, tsl], ps, ctk[:, 0, :], ctk[:, 1, :], 64)
                nc.scalar.copy(k2[64:128, tsl], k2[0:64, tsl])

                # V^T -> V via HWDGE xbar transpose (keeps the PE free)
                vt = work.tile([64, TC], bf16, tag="vt", name="vt")
                nc.scalar.copy(vt[:], ps[64:128, :])
                for j in range(TC // KB):
                    kbi = (TC // KB) * tc8 + j
                    teng = nc.sync if j % 2 == 0 else nc.scalar
                    teng.dma_start_transpose(vatt[kbi][:, 0:D],
                                             vt[:, 128 * j:128 * (j + 1)])

            # mask + ones-columns: only read from attention on; emitting them
            # here keeps the startup DMA queues clear for x/weights.
            nc.gpsimd.dma_start(msq[:], mq_d[:])
            for i in range(T // KB):
                nc.gpsimd.memset(vatt[i][:, D:D + 1], 1.0)

            # ---- wo prefetch: runs on DMA queues during attention ----
            # nt 0/1 get persistent tiles; nt 2/3 reuse the freed x-stream
            # slots (same [128, 16, 512] bf16 shape, x is done after phase 1).
            wo_sb = []
            w_eng = (nc.sync, nc.scalar, nc.sync, nc.scalar)
            for nt in range(4):
                if nt < 2:
                    t = persist.tile([128, 16, 512], bf16, tag=f"wo{nt}",
                                     name=f"wo{nt}")
                else:
                    t = stream.tile([128, 16, 512], bf16, tag="s", name=f"wo{nt}")
                w_eng[nt].dma_start(t[:], wo[nt])
                wo_sb.append(t)

            # ================= Attention (head-pair parallel, causal) ==========
            a2a_in = [dram.tile([NC, 130, TL], bf16, tag=f"a2a_in{t}",
                                name=f"a2a_in{t}") for t in range(2)]
            a2a_out = [dram.tile([NC, 130, TL], bf16, tag=f"a2a_out{t}",
                                 name=f"a2a_out{t}") for t in range(2)]
            den_sb = [persist.tile([2 * NC, TC], bf16, tag=f"den{t}",
                                   name=f"den{t}") for t in range(2)]
            ao = {}
            mask_ctr = 0
            exp_acc = 0.0

            def attn_pair(pair):
                nonlocal mask_ctr, exp_acc
                qtile = qT[pair]
                for b in range(B):
                    for cq in range(4):
                        nkb = 4 * (cq + 1)
                        qs = S * b + TC * cq
                        j = 4 * b + cq          # destination core for this unit
                        psOa = psum.tile([128, 512], fp32, tag="mm", bufs=2,
                                         name="psOa")[0:D + 1, :]
                        psOb = psum.tile([128, 512], fp32, tag="mm", bufs=2,
                                         name="psOb")[0:D + 1, :]
                        for kb in range(nkb):
                            dj = kb - (nkb - 4)  # >=0: diagonal-band index
                            qoff = 128 * dj if (DIAG_TRIM and dj > 0) else 0
                            kpos = S * b + KB * kb
                            psS = psum.tile([128, 2 * TC], fp32, tag="big",
                                            bufs=3, name="psS")
                            ex = work.tile([128, 2 * TC], bf16, tag="ex", bufs=6,
                                           name="ex")
                            for h in range(2):
                                nc.tensor.matmul(
                                    psS[:, TC * h + qoff:TC * (h + 1)],
                                    k2[64 * h:64 * (h + 1), kpos:kpos + KB],
                                    qtile[64 * h:64 * (h + 1),
                                          qs + qoff:qs + TC],
                                    start=True, stop=True)
                            ps2 = psS.rearrange("p (h c) -> p h c", h=2)
                            ex2 = ex.rearrange("p (h c) -> p h c", h=2)
                            exi = ex.bitcast(i16).rearrange("p (h c) -> p h c", h=2)
                            exp_acc += DVE_EXP
                            if exp_acc >= 1.0:
                                exp_acc -= 1.0
                                nc.vector.tensor_scalar(
                                    exi[:, :, qoff:TC], ps2[:, :, qoff:TC],
                                    SCHR_A, SCHR_B,
                                    mybir.AluOpType.mult, mybir.AluOpType.add)
                            else:
                                nc.scalar.activation(
                                    ex2[:, :, qoff:TC], ps2[:, :, qoff:TC],
                                    mybir.ActivationFunctionType.Exp)
                            if dj >= 0:
                                # only the 128x128 diagonal square needs masking
                                meng = nc.gpsimd if (mask_ctr % 2 == 0) else nc.vector
                                mask_ctr += 1
                                meng.tensor_mul(ex2[:, :, qoff:qoff + 128],
                                                ex2[:, :, qoff:qoff + 128],
                                                msq[:, :, :])
                            for h, psO in ((0, psOa), (1, psOb)):
                                nc.tensor.matmul(
                                    psO[:, qoff:TC],
                                    vatt[(S // KB) * b + kb][:],
                                    ex2[:, h, qoff:TC],
                                    start=(kb == 0), stop=(kb == nkb - 1))
                        # drain: attn rows straight to a2a staging; den row to
                        # the local den staging tile (reciprocal'd pre-a2a)
                        for h, psO in ((0, psOa), (1, psOb)):
                            bounce = work.tile([D + 1, 512], bf16, tag="bounce",
                                               bufs=6, name="bounce")
                            if h == 0:
                                nc.scalar.copy(bounce[:], psO[:])
                            else:
                                nc.vector.tensor_copy(bounce[:], psO[:])
                            nc.sync.dma_start(
                                a2a_in[pair][j, 65 * h:65 * h + D, :],
                                bounce[0:D, :])
                            nc.sync.dma_start(
                                den_sb[pair][2 * j + h:2 * j + h + 1, :],
                                bounce[D:D + 1, :])

            i32 = mybir.dt.int32
            RMAGIC = 0x7EF127EA

            def recip_stage(pair):
                # 1/den on GpSimd (float bit-trick + 2 Newton steps) so neither
                # the Scalar nor Vector queue is touched; the reciprocals ride
                # the a2a in the payload's den rows (64/129 of each slot).
                ds = work.tile([2 * NC, TC], fp32, tag="rc_ds", bufs=1, name="ds")
                nc.gpsimd.tensor_copy(ds[:], den_sb[pair][:])      # bf16 -> f32
                bf = work.tile([2 * NC, TC], fp32, tag="rc_bf", bufs=1, name="bf")
                nc.gpsimd.tensor_copy(bf[:], ds.bitcast(i32)[:])   # bits as f32
                nc.gpsimd.tensor_scalar(bf[:], bf[:], -1.0, float(RMAGIC),
                                        mybir.AluOpType.mult,
                                        mybir.AluOpType.add)
                r0i = work.tile([2 * NC, TC], i32, tag="r0i", bufs=1, name="r0i")
                nc.gpsimd.tensor_copy(r0i[:], bf[:])               # back to bits
                r0 = r0i.bitcast(fp32)
                t = work.tile([2 * NC, TC], fp32, tag="rc_t", bufs=1, name="rc_t")
                for _ in range(2):
                    nc.gpsimd.tensor_mul(t[:], ds[:], r0[:])
                    nc.gpsimd.tensor_scalar(t[:], t[:], -1.0, 2.0,
                                            mybir.AluOpType.mult,
                                            mybir.AluOpType.add)
                    nc.gpsimd.tensor_mul(r0[:], r0[:], t[:])
                rb = work.tile([2 * NC, TC], bf16, tag="rcb", bufs=1, name="rcb")
                nc.gpsimd.tensor_copy(rb[:], r0[:])
                nc.gpsimd.dma_start(
                    a2a_in[pair].rearrange("a (x b) c -> (a x) b c", x=2)[:, 64, :],
                    rb[:])

            def a2a_post(pair):
                # payload den rows already hold 1/den (bf16): broadcast straight
                # from the a2a output in DRAM and multiply into the persistent
                # ao tile.  All DMA targets use the "bounce"-tag ring so the
                # scheduler sees a real dependency on late pair-1 work and
                # cannot queue these ahead of attention (a blocked queue head
                # would starve the whole machine while the a2a runs).  Pair 0
                # uses the sync queue, pair 1 the gpsimd queue, so a pair-1
                # DMA blocked on the second collective can never delay pair-0
                # work that is already ready.
                a2a_v = a2a_out[pair].rearrange("a (x b) c -> (a x) b c", x=2)
                for r in range(NC):
                    kk = 2 * r + pair
                    # pair 0 spreads across sync+scalar (both idle once
                    # pair-1's attention winds down); pair 1 stays entirely on
                    # gpsimd so its blocked DMAs can't stall anything else.
                    if pair:
                        # scalar is idle once passA's drains finish, and a
                        # blocked post-1 DMA there can't stall anything that
                        # matters (everything later also needs the a2a)
                        deng = nc.gpsimd if r % 2 == 0 else nc.scalar
                        deng2 = nc.scalar if r % 2 == 0 else nc.gpsimd
                    else:
                        deng = nc.sync if r % 2 == 0 else nc.scalar
                        deng2 = nc.scalar if r % 2 == 0 else nc.sync
                    tl_ = persist.tile([128, TL], bf16, tag=f"ao{kk}",
                                       name=f"ao{kk}")
                    w = work.tile([128, TL], bf16, tag="bounce", bufs=6,
                                  name="aow")
                    rb2 = work.tile([128, TL], bf16, tag="bounce", bufs=6,
                                    name="rb2")
                    deng.dma_start(w[0:64, :], a2a_out[pair][r, 0:64, :])
                    deng2.dma_start(w[64:128, :], a2a_out[pair][r, 65:129, :])
                    deng.dma_start(
                        rb2[0:64, :],
                        a2a_v[2 * r:2 * r + 1, 64, :].broadcast_to([64, TL]))
                    deng2.dma_start(
                        rb2[64:128, :],
                        a2a_v[2 * r + 1:2 * r + 2, 64, :].broadcast_to([64, TL]))
                    nc.vector.tensor_mul(tl_[:], w[:], rb2[:])
                    ao[kk] = tl_

            def a2a_go(pair):
                nc.gpsimd.collective_compute(
                    "AllToAll", mybir.AluOpType.bypass,
                    replica_groups=[list(range(NC))],
                    ins=[a2a_in[pair].opt()], outs=[a2a_out[pair].opt()])

            # pair-0 post-collective work is emitted AFTER pair-1's attention
            # so the FIFO engine queues never make pair-1's attention wait on
            # the a2a (the ring-gated DMA targets enforce late placement).
            attn_pair(0)
            recip_stage(0)
            a2a_go(0)
            attn_pair(1)
            recip_stage(1)
            a2a_go(1)
            a2a_post(0)

            # ================= Phase 2: output projection =====================
            # Two passes over the contraction: pass A (kk even — pair-0 ao
            # tiles only) runs for ALL 16 output groups while the pair-1
            # collective is still in flight, draining half-sums to bf16
            # partials; pass B (kk odd) adds the partials back in.
            def ps_slot(pg):
                if pg % 5 < 3:
                    return psum.tile([128, 2 * TC], fp32, tag="big", bufs=3,
                                     name="ps_o")[:, 0:512]
                return psum.tile([128, 512], fp32, tag="mm", bufs=2,
                                 name="ps_o")

            prt = {}
            pg = 0
            for nt in range(4):
                for tt in range(TL // 128):
                    ps = ps_slot(pg)
                    pg += 1
                    for ki in range(8):
                        kk = 2 * ki
                        nc.tensor.matmul(ps[:], ao[kk][:, 128 * tt:128 * (tt + 1)],
                                         wo_sb[nt][:, kk, :],
                                         start=(ki == 0), stop=(ki == 7))
                    p = work.tile([128, 512], bf16, tag=f"prt{nt}{tt}", bufs=1,
                                  name="prt")
                    if (nt + tt) % 2 == 0:
                        nc.scalar.copy(p[:], ps[:])
                    else:
                        nc.vector.tensor_copy(p[:], ps[:])
                    prt[(nt, tt)] = p

            a2a_post(1)

            for nt in range(4):
                for tt in range(TL // 128):
                    ps = ps_slot(pg)
                    pg += 1
                    for ki in range(8):
                        kk = 2 * ki + 1
                        nc.tensor.matmul(ps[:], ao[kk][:, 128 * tt:128 * (tt + 1)],
                                         wo_sb[nt][:, kk, :],
                                         start=(ki == 0), stop=(ki == 7))
                    ob = work.tile([128, 512], fp32, tag="ob", bufs=2, name="ob")
                    last = (nt == 3)
                    if (nt + tt) % 2 == 0 and not last:
                        # keep the adds off the busy Vector queue: Scalar
                        # evacuates PSUM, GpSimd does the add in SBUF
                        obf = work.tile([128, 512], fp32, tag="obf", bufs=2,
                                        name="obf")
                        nc.scalar.copy(obf[:], ps[:])
                        nc.gpsimd.tensor_add(ob[:], obf[:], prt[(nt, tt)][:])
                    else:
                        nc.vector.tensor_add(ob[:], ps[:], prt[(nt, tt)][:])
                    oeng = nc.gpsimd if ((nt + tt) % 2 == 0 and not last) \
                        else nc.sync
                    oeng.dma_start(out_d[128 * tt:128 * (tt + 1),
                                         512 * nt:512 * (nt + 1)], ob[:])

    nc.compile()
    return nc


def _prep_inputs(x, cos, sin, wq, wk, wv, wo):
    x = np.asarray(x, F32)
    cos = np.asarray(cos, F32)
    sin = np.asarray(sin, F32)
    wq = np.asarray(wq, F32)
    wk = np.asarray(wk, F32)
    wv = np.asarray(wv, F32)
    wo = np.asarray(wo, F32)

    # x pre-arranged so each phase-1 chunk load is one fully-contiguous DMA:
    # xprep[t8, p, k, c] = x.T[128k + p, 512 t8 + c]
    xT2 = np.ascontiguousarray(x.reshape(T, HID).T).astype(BF16)
    xprep = np.ascontiguousarray(
        xT2.reshape(16, 128, NTC, TC).transpose(2, 1, 0, 3))    # [8,128,16,512]
    # wo likewise: woprep[nt, p, k, n] = wo[128k + p, 512 nt + n]
    woprep = np.ascontiguousarray(
        wo.astype(BF16).reshape(16, 128, 4, 512).transpose(2, 1, 0, 3))

    pos = np.arange(T) % S
    sign = np.concatenate([-np.ones(D // 2, F32), np.ones(D // 2, F32)])
    ctk = np.ascontiguousarray(cos[pos].T)                      # [64, T]
    stk = np.ascontiguousarray((sin[pos] * sign).T)             # [64, T]
    ctk2 = np.ascontiguousarray(np.stack([ctk, stk], 1))        # [64, 2, T]
    scale = F32(1.0 / np.sqrt(D))
    ctq2 = np.ascontiguousarray(
        np.stack([np.concatenate([ctk, ctk], 0) * scale,
                  np.concatenate([stk, stk], 0) * scale], 1))   # [128, 2, T]

    # single 128x128 lower-tri mask, doubled for the two heads of a pair
    kl = np.arange(128)
    msq = (kl[None, :] >= kl[:, None]).astype(BF16)
    mq = np.ascontiguousarray(np.stack([msq, msq], 1))          # [128, 2, 128]

    in_maps = []
    for c in range(NC):
        wq_cc = np.ascontiguousarray(
            wq[:, c * LH * D:(c + 1) * LH * D].astype(BF16)
            .reshape(16, 128, LH * D).transpose(1, 0, 2))       # [128,16,256]
        wkv_cc = np.ascontiguousarray(
            np.concatenate([wk[:, c * D:(c + 1) * D],
                            wv[:, c * D:(c + 1) * D]], 1).astype(BF16)
            .reshape(16, 128, 2 * D).transpose(1, 0, 2))        # [128,16,128]
        in_maps.append({
            "xT": xprep, "wq_c": wq_cc, "wkv_c": wkv_cc, "wo": woprep,
            "ctq": ctq2, "ctk": ctk2, "mq": mq,
        })
    return in_maps


def get_nc():
    if "nc" not in _CACHE:
        _CACHE["nc"] = _build()
    return _CACHE["nc"]


def run(in_maps, **kwargs):
    nc = get_nc()
    return run_bass_kernel_spmd(nc, in_maps, core_ids=list(range(NC)), **kwargs)


def kernel(x, cos, sin, wq, wk, wv, wo):
    in_maps = _prep_inputs(x, cos, sin, wq, wk, wv, wo)
    res = run(in_maps)
    out = np.empty((T, HID), F32)
    for c in range(NC):
        out[TL * c:TL * (c + 1)] = res.results[c]["out"]
    return out.reshape(B, S, HID)


# revision 57
# speedup vs baseline: 1.0283x; 1.0283x over previous
"""Distributed Bass kernel for nn_Attention (B=2, S=2048, HID=2048, H=32, KVH=8, D=64).

Sharding (8 NeuronCores, uniform SPMD graph):
  - Head-parallel phase: core c owns kv-head c and its 4 GQA query heads.
    x replicated as xT [HID, B*S] bf16; per-core Q^T [256, 4096] (+RoPE,
    1/sqrt(D) folded into trig tables), K^T [64, 4096] (+RoPE, duplicated to
    rows 64:128), V [keys, 64|1] blocks with a ones-column for the softmax
    denominator.
  - Attention in S^T layout [keys, queries], processed per head-PAIR: the two
    heads of a pair sit on PE row-groups 0/64, so their K=64 S^T matmuls run
    concurrently (row-group tiling).  exp is split between the Scalar engine
    (true Exp) and the Vector engine (one-instruction Schraudolph bf16
    bit-trick exp ~ bitcast(int16(x*128/ln2 + 16248.75))) so neither engine
    bottlenecks.  Causal masking: rectangular blocks need no mask; the 4
    diagonal-band blocks per chunk use width-trimmed matmuls plus a 0/1 mask
    multiply (split Vector/GpSimd).
  - Per-unit drains DMA straight into the AllToAll staging buffer
    [NC, 130, TL] whose rows 64/129 carry the (bf16) softmax denominators —
    one collective per head-pair, no separate denominator collective.  Pair 0's
    collective overlaps pair 1's attention.
  - Token-parallel output projection: ao tiles normalized post-collective,
    out rows = ao.T @ wo per 512-col block; wo prefetched during attention.
"""

import os
import numpy as np
import ml_dtypes

import concourse.bass as bass
import concourse.mybir as mybir
import concourse.tile as tile
from concourse import bacc
from concourse.bass_utils import run_bass_kernel_spmd

BF16 = ml_dtypes.bfloat16
F32 = np.float32

B, S, HID = 2, 2048, 2048
H, KVH, D = 32, 8, 64
NC = 8                 # cores
T = B * S              # 4096 flat tokens
TL = T // NC           # 512 tokens per core (phase-2 output rows)
LH = H // NC           # 4 local q-heads per core
KB = 128               # key block
TC = 512               # phase-1 token streaming chunk / query chunk
NTC = T // TC          # 8 token chunks

SCHR_A = float(128.0 / np.log(2.0))
SCHR_B = float(127.0 * 128.0 - 7.25)

_CACHE = {}


def _build():
    fp32 = mybir.dt.float32
    bf16 = mybir.dt.bfloat16
    i16 = mybir.dt.int16

    # knobs (read at build time; defaults are the tuned config)
    DVE_EXP = float(os.environ.get("KDVE", "0.5"))   # fraction of exps on DVE
    DIAG_TRIM = os.environ.get("KDIAG", "1") == "1"  # width-trim diagonal blocks

    fp8 = mybir.dt.float8e4
    DR = mybir.MatmulPerfMode.DoubleRow

    nc = bacc.Bacc("TRN2", target_bir_lowering=False, debug=False, num_devices=NC)

    xT = nc.dram_tensor("xT", [NTC, 128, 16, TC], bf16, kind="ExternalInput")
    wq_c = nc.dram_tensor("wq_c", [128, 16, LH * D], bf16, kind="ExternalInput")
    wkv_c = nc.dram_tensor("wkv_c", [128, 16, 2 * D], bf16, kind="ExternalInput")
    wo = nc.dram_tensor("wo", [4, 128, 16, 512], bf16, kind="ExternalInput")
    ctq_d = nc.dram_tensor("ctq", [128, 2, T], fp32, kind="ExternalInput")
    ctk_d = nc.dram_tensor("ctk", [64, 2, T], fp32, kind="ExternalInput")
    mq_d = nc.dram_tensor("mq", [128, 2, 128], bf16, kind="ExternalInput")
    out_d = nc.dram_tensor("out", [TL, HID], fp32, kind="ExternalOutput")

    with tile.TileContext(nc) as tc:
        with (
            tc.tile_pool(name="persist", bufs=1) as persist,
            tc.tile_pool(name="stream", bufs=2) as stream,
            tc.tile_pool(name="trig", bufs=2) as trig,
            tc.tile_pool(name="work", bufs=2) as work,
            tc.tile_pool(name="psum", bufs=1, space="PSUM") as psum,
            tc.tile_pool(name="dram", bufs=1, space="DRAM") as dram,
        ):
            # ---- persistent tiles ----
            qT = [persist.tile([128, T], bf16, tag=f"qT{t}", name=f"qT{t}")
                  for t in range(2)]
            k2 = persist.tile([128, T], bf16, tag="k2", name="k2")
            vatt = [persist.tile([128, D + 1], bf16, tag=f"vatt{i}", name=f"vatt{i}")
                    for i in range(T // KB)]
            msq = persist.tile([128, 2, 128], bf16, tag="msq", name="msq")
            ident = persist.tile([128, 128], bf16, tag="ident", name="ident")

            # ---- weights (merged single-DMA loads) ----
            wq_sb = persist.tile([128, 16, LH * D], bf16, tag="wq", name="wq")
            wkv_sb = persist.tile([128, 16, 2 * D], bf16, tag="wkv", name="wkv")
            for g in range(4):
                nc.scalar.dma_start(wq_sb[:, 4 * g:4 * (g + 1), :],
                                    wq_c[:, 4 * g:4 * (g + 1), :])
            nc.scalar.dma_start(wkv_sb[:], wkv_c[:])

            from concourse.masks import make_identity
            make_identity(nc, ident[:])

            # ================= Phase 1: QKV projections + RoPE =================
            def rope(out_ap, ps, ct, st, npart):
                """out = ps*ct + swap32(ps)*st  (st carries the rotate-half sign)."""
                t1 = work.tile([128, TC], fp32, tag="rope_t1", name="t1")
                t2 = work.tile([128, TC], fp32, tag="rope_t2", name="t2")
                nc.vector.tensor_mul(t1[:npart, :], ps[:npart, :], ct[:npart, :])
                for base in range(0, npart, 64):
                    a, b2 = base, base + 32
                    nc.vector.tensor_mul(t2[a:a + 32, :], ps[b2:b2 + 32, :], st[a:a + 32, :])
                    nc.vector.tensor_mul(t2[b2:b2 + 32, :], ps[a:a + 32, :], st[b2:b2 + 32, :])
                nc.vector.tensor_add(out_ap, t1[:npart, :], t2[:npart, :])

            for tc8 in range(NTC):
                tsl = slice(TC * tc8, TC * (tc8 + 1))
                xt = stream.tile([128, 16, TC], bf16, tag="s", name=f"x{tc8}")
                if tc8 == 0:
                    # split the first chunk so the first matmuls start early
                    for g in range(4):
                        nc.sync.dma_start(xt[:, 4 * g:4 * (g + 1), :],
                                          xT[tc8, :, 4 * g:4 * (g + 1), :])
                elif tc8 in (1, 2):
                    # gpsimd is idle at startup: prefetch chunks 1-2 there so
                    # they don't queue behind chunk 0's pieces
                    nc.gpsimd.dma_start(xt[:], xT[tc8])
                else:
                    nc.sync.dma_start(xt[:, 0:8, :], xT[tc8, :, 0:8, :])
                    nc.scalar.dma_start(xt[:, 8:16, :], xT[tc8, :, 8:16, :])
                ctq = trig.tile([128, 2, TC], fp32, tag="ctq", name="ctq")
                ctk = trig.tile([64, 2, TC], fp32, tag="ctk", name="ctk")
                # chunk 0's tables ride the idle gpsimd queue at startup so
                # they don't wait behind the weight loads on scalar
                teng = nc.gpsimd if tc8 == 0 else nc.scalar
                teng.dma_start(ctq[:], ctq_d[:, :, tsl])
                teng.dma_start(ctk[:], ctk_d[:, :, tsl])

                # Q^T: two 128-row tiles (2 heads each)
                for qt in range(2):
                    ps = psum.tile([128, 2 * TC], fp32, tag="big", bufs=3,
                                   name="ps_q")[:, 0:TC]
                    for k in range(16):
                        nc.tensor.matmul(ps[:], wq_sb[:, k, 128 * qt:128 * (qt + 1)],
                                         xt[:, k, :], start=(k == 0), stop=(k == 15))
                    rope(qT[qt][:, tsl], ps, ctq[:, 0, :], ctq[:, 1, :], 128)

                # K^T (rows 0:64) and V^T (rows 64:128) in one packed projection
                ps = psum.tile([128, 2 * TC], fp32, tag="big", bufs=3,
                               name="ps_kv")[:, 0:TC]
                for k in range(16):
                    nc.tensor.matmul(ps[:], wkv_sb[:, k, :], xt[:, k, :],
                                     start=(k == 0), stop=(k == 15))
                rope(k2[0:64, tsl], ps, ctk[:, 0, :], ctk[:, 1, :], 64)
                nc.scalar.copy(k2[64:128, tsl], k2[0:64, tsl])

                # V^T -> V via HWDGE xbar transpose (keeps the PE free)
                vt = work.tile([64, TC], bf16, tag="vt", name="vt")
                nc.scalar.copy(vt[:], ps[64:128, :])
                for j in range(TC // KB):
                    kbi = (TC // KB) * tc8 + j
                    teng = nc.sync if j % 2 == 0 else nc.scalar
                    teng.dma_start_transpose(vatt[kbi][:, 0:D],
                                             vt[:, 128 * j:128 * (j + 1)])

            # mask + ones-columns: only read from attention on; emitting them
            # here keeps the startup DMA queues clear for x/weights.
            nc.gpsimd.dma_start(msq[:], mq_d[:])
            for i in range(T // KB):
                nc.gpsimd.memset(vatt[i][:, D:D + 1], 1.0)

            # ---- wo prefetch: runs on DMA queues during attention ----
            # nt 0/1 get persistent tiles; nt 2/3 reuse the freed x-stream
            # slots (same [128, 16, 512] bf16 shape, x is done after phase 1).
            wo_sb = []
            w_eng = (nc.sync, nc.scalar, nc.sync, nc.scalar)
            for nt in range(4):
                if nt < 2:
                    t = persist.tile([128, 16, 512], bf16, tag=f"wo{nt}",
                                     name=f"wo{nt}")
                else:
                    t = stream.tile([128, 16, 512], bf16, tag="s", name=f"wo{nt}")
                w_eng[nt].dma_start(t[:], wo[nt])
                wo_sb.append(t)

            # ================= Attention (head-pair parallel, causal) ==========
            a2a_in = [dram.tile([NC, 130, TL], bf16, tag=f"a2a_in{t}",
                                name=f"a2a_in{t}") for t in range(2)]
            a2a_out = [dram.tile([NC, 130, TL], bf16, tag=f"a2a_out{t}",
                                 name=f"a2a_out{t}") for t in range(2)]
            den_sb = [persist.tile([2 * NC, TC], bf16, tag=f"den{t}",
                                   name=f"den{t}") for t in range(2)]
            ao = {}
            mask_ctr = 0
            exp_acc = 0.0

            def attn_pair(pair):
                nonlocal mask_ctr, exp_acc
                qtile = qT[pair]
                for b in range(B):
                    for cq in range(4):
                        nkb = 4 * (cq + 1)
                        qs = S * b + TC * cq
                        j = 4 * b + cq          # destination core for this unit
                        psOa = psum.tile([128, 512], fp32, tag="mm", bufs=2,
                                         name="psOa")[0:D + 1, :]
                        psOb = psum.tile([128, 512], fp32, tag="mm", bufs=2,
                                         name="psOb")[0:D + 1, :]
                        for kb in range(nkb):
                            dj = kb - (nkb - 4)  # >=0: diagonal-band index
                            qoff = 128 * dj if (DIAG_TRIM and dj > 0) else 0
                            kpos = S * b + KB * kb
                            psS = psum.tile([128, 2 * TC], fp32, tag="big",
                                            bufs=3, name="psS")
                            ex = work.tile([128, 2 * TC], bf16, tag="ex", bufs=6,
                                           name="ex")
                            for h in range(2):
                                nc.tensor.matmul(
                                    psS[:, TC * h + qoff:TC * (h + 1)],
                                    k2[64 * h:64 * (h + 1), kpos:kpos + KB],
                                    qtile[64 * h:64 * (h + 1),
                                          qs + qoff:qs + TC],
                                    start=True, stop=True)
                            ps2 = psS.rearrange("p (h c) -> p h c", h=2)
                            ex2 = ex.rearrange("p (h c) -> p h c", h=2)
                            exi = ex.bitcast(i16).rearrange("p (h c) -> p h c", h=2)
                            exp_acc += DVE_EXP
                            if exp_acc >= 1.0:
                                exp_acc -= 1.0
                                nc.vector.tensor_scalar(
                                    exi[:, :, qoff:TC], ps2[:, :, qoff:TC],
                                    SCHR_A, SCHR_B,
                                    mybir.AluOpType.mult, mybir.AluOpType.add)
                            else:
                                nc.scalar.activation(
                                    ex2[:, :, qoff:TC], ps2[:, :, qoff:TC],
                                    mybir.ActivationFunctionType.Exp)
                            if dj >= 0:
                                # only the 128x128 diagonal square needs masking
                                meng = nc.gpsimd if (mask_ctr % 2 == 0) else nc.vector
                                mask_ctr += 1
                                meng.tensor_mul(ex2[:, :, qoff:qoff + 128],
                                                ex2[:, :, qoff:qoff + 128],
                                                msq[:, :, :])
                            for h, psO in ((0, psOa), (1, psOb)):
                                nc.tensor.matmul(
                                    psO[:, qoff:TC],
                                    vatt[(S // KB) * b + kb][:],
                                    ex2[:, h, qoff:TC],
                                    start=(kb == 0), stop=(kb == nkb - 1))
                        # drain: attn rows straight to a2a staging; den row to
                        # the local den staging tile (reciprocal'd pre-a2a)
                        for h, psO in ((0, psOa), (1, psOb)):
                            bounce = work.tile([D + 1, 512], bf16, tag="bounce",
                                               bufs=6, name="bounce")
                            if h == 0:
                                nc.scalar.copy(bounce[:], psO[:])
                            else:
                                nc.vector.tensor_copy(bounce[:], psO[:])
                            nc.sync.dma_start(
                                a2a_in[pair][j, 65 * h:65 * h + D, :],
                                bounce[0:D, :])
                            nc.sync.dma_start(
                                den_sb[pair][2 * j + h:2 * j + h + 1, :],
                                bounce[D:D + 1, :])

            i32 = mybir.dt.int32
            RMAGIC = 0x7EF127EA

            def recip_stage(pair):
                # 1/den on GpSimd (float bit-trick + 2 Newton steps) so neither
                # the Scalar nor Vector queue is touched; the reciprocals ride
                # the a2a in the payload's den rows (64/129 of each slot).
                ds = work.tile([2 * NC, TC], fp32, tag="rc_ds", bufs=1, name="ds")
                nc.gpsimd.tensor_copy(ds[:], den_sb[pair][:])      # bf16 -> f32
                bf = work.tile([2 * NC, TC], fp32, tag="rc_bf", bufs=1, name="bf")
                nc.gpsimd.tensor_copy(bf[:], ds.bitcast(i32)[:])   # bits as f32
                nc.gpsimd.tensor_scalar(bf[:], bf[:], -1.0, float(RMAGIC),
                                        mybir.AluOpType.mult,
                                        mybir.AluOpType.add)
                r0i = work.tile([2 * NC, TC], i32, tag="r0i", bufs=1, name="r0i")
                nc.gpsimd.tensor_copy(r0i[:], bf[:])               # back to bits
                r0 = r0i.bitcast(fp32)
                t = work.tile([2 * NC, TC], fp32, tag="rc_t", bufs=1, name="rc_t")
                for _ in range(2):
                    nc.gpsimd.tensor_mul(t[:], ds[:], r0[:])
                    nc.gpsimd.tensor_scalar(t[:], t[:], -1.0, 2.0,
                                            mybir.AluOpType.mult,
                                            mybir.AluOpType.add)
                    nc.gpsimd.tensor_mul(r0[:], r0[:], t[:])
                rb = work.tile([2 * NC, TC], bf16, tag="rcb", bufs=1, name="rcb")
                nc.gpsimd.tensor_copy(rb[:], r0[:])
                nc.gpsimd.dma_start(
                    a2a_in[pair].rearrange("a (x b) c -> (a x) b c", x=2)[:, 64, :],
                    rb[:])

            def a2a_post(pair):
                # payload den rows already hold 1/den (bf16): broadcast straight
                # from the a2a output in DRAM and multiply into the persistent
                # ao tile.  All DMA targets use the "bounce"-tag ring so the
                # scheduler sees a real dependency on late pair-1 work and
                # cannot queue these ahead of attention (a blocked queue head
                # would starve the whole machine while the a2a runs).  Pair 0
                # uses the sync queue, pair 1 the gpsimd queue, so a pair-1
                # DMA blocked on the second collective can never delay pair-0
                # work that is already ready.
                a2a_v = a2a_out[pair].rearrange("a (x b) c -> (a x) b c", x=2)
                for r in range(NC):
                    kk = 2 * r + pair
                    # pair 0 spreads across sync+scalar (both idle once
                    # pair-1's attention winds down); pair 1 stays entirely on
                    # gpsimd so its blocked DMAs can't stall anything else.
                    if pair:
                        # scalar is idle once passA's drains finish, and a
                        # blocked post-1 DMA there can't stall anything that
                        # matters (everything later also needs the a2a)
                        deng = nc.gpsimd if r % 2 == 0 else nc.scalar
                        deng2 = nc.scalar if r % 2 == 0 else nc.gpsimd
                    else:
                        deng = nc.sync if r % 2 == 0 else nc.scalar
                        deng2 = nc.scalar if r % 2 == 0 else nc.sync
                    tl_ = persist.tile([128, TL], bf16, tag=f"ao{kk}",
                                       name=f"ao{kk}")
                    w = work.tile([128, TL], bf16, tag="bounce", bufs=6,
                                  name="aow")
                    rb2 = work.tile([128, TL], bf16, tag="bounce", bufs=6,
                                    name="rb2")
                    deng.dma_start(w[0:64, :], a2a_out[pair][r, 0:64, :])
                    deng2.dma_start(w[64:128, :], a2a_out[pair][r, 65:129, :])
                    deng.dma_start(
                        rb2[0:64, :],
                        a2a_v[2 * r:2 * r + 1, 64, :].broadcast_to([64, TL]))
                    deng2.dma_start(
                        rb2[64:128, :],
                        a2a_v[2 * r + 1:2 * r + 2, 64, :].broadcast_to([64, TL]))
                    nc.vector.tensor_mul(tl_[:], w[:], rb2[:])
                    ao[kk] = tl_

            def a2a_go(pair):
                nc.gpsimd.collective_compute(
                    "AllToAll", mybir.AluOpType.bypass,
                    replica_groups=[list(range(NC))],
                    ins=[a2a_in[pair].opt()], outs=[a2a_out[pair].opt()])

            # pair-0 post-collective work is emitted AFTER pair-1's attention
            # so the FIFO engine queues never make pair-1's attention wait on
            # the a2a (the ring-gated DMA targets enforce late placement).
            attn_pair(0)
            recip_stage(0)
            a2a_go(0)
            attn_pair(1)
            recip_stage(1)
            a2a_go(1)
            a2a_post(0)

            # ================= Phase 2: output projection =====================
            # Two passes over the contraction: pass A (kk even — pair-0 ao
            # tiles only) runs for ALL 16 output groups while the pair-1
            # collective is still in flight, draining half-sums to bf16
            # partials; pass B (kk odd) adds the partials back in.
            def ps_slot(pg):
                if pg % 5 < 3:
                    return psum.tile([128, 2 * TC], fp32, tag="big", bufs=3,
                                     name="ps_o")[:, 0:512]
                return psum.tile([128, 512], fp32, tag="mm", bufs=2,
                                 name="ps_o")

            prt = {}
            pg = 0
            for nt in range(4):
                for tt in range(TL // 128):
                    ps = ps_slot(pg)
                    pg += 1
                    for ki in range(8):
                        kk = 2 * ki
                        nc.tensor.matmul(ps[:], ao[kk][:, 128 * tt:128 * (tt + 1)],
                                         wo_sb[nt][:, kk, :],
                                         start=(ki == 0), stop=(ki == 7))
                    p = work.tile([128, 512], bf16, tag=f"prt{nt}{tt}", bufs=1,
                                  name="prt")
                    if (nt + tt) % 2 == 0:
                        nc.scalar.copy(p[:], ps[:])
                    else:
                        nc.vector.tensor_copy(p[:], ps[:])
                    prt[(nt, tt)] = p

            a2a_post(1)

            for nt in range(4):
                for tt in range(TL // 128):
                    ps = ps_slot(pg)
                    pg += 1
                    for ki in range(8):
                        kk = 2 * ki + 1
                        nc.tensor.matmul(ps[:], ao[kk][:, 128 * tt:128 * (tt + 1)],
                                         wo_sb[nt][:, kk, :],
                                         start=(ki == 0), stop=(ki == 7))
                    ob = work.tile([128, 512], fp32, tag="ob", bufs=2, name="ob")
                    last = (nt == 3)
                    if (nt + tt) % 2 == 0 and not last:
                        # keep the adds off the busy Vector queue: Scalar
                        # evacuates PSUM, GpSimd does the add in SBUF
                        obf = work.tile([128, 512], fp32, tag="obf", bufs=2,
                                        name="obf")
                        nc.scalar.copy(obf[:], ps[:])
                        nc.gpsimd.tensor_add(ob[:], obf[:], prt[(nt, tt)][:])
                    else:
                        nc.vector.tensor_add(ob[:], ps[:], prt[(nt, tt)][:])
                    oeng = nc.gpsimd if ((nt + tt) % 2 == 0 and not last) \
                        else nc.sync
                    oeng.dma_start(out_d[128 * tt:128 * (tt + 1),
                                         512 * nt:512 * (nt + 1)], ob[:])

    nc.compile()
    return nc


def _prep_inputs(x, cos, sin, wq, wk, wv, wo):
    x = np.asarray(x, F32)
    cos = np.asarray(cos, F32)
    sin = np.asarray(sin, F32)
    wq = np.asarray(wq, F32)
    wk = np.asarray(wk, F32)
    wv = np.asarray(wv, F32)
    wo = np.asarray(wo, F32)

    # x pre-arranged so each phase-1 chunk load is one fully-contiguous DMA:
    # xprep[t8, p, k, c] = x.T[128k + p, 512 t8 + c]
    xT2 = np.ascontiguousarray(x.reshape(T, HID).T).astype(BF16)
    xprep = np.ascontiguousarray(
        xT2.reshape(16, 128, NTC, TC).transpose(2, 1, 0, 3))    # [8,128,16,512]
    # wo likewise: woprep[nt, p, k, n] = wo[128k + p, 512 nt + n]
    woprep = np.ascontiguousarray(
        wo.astype(BF16).reshape(16, 128, 4, 512).transpose(2, 1, 0, 3))

    pos = np.arange(T) % S
    sign = np.concatenate([-np.ones(D // 2, F32), np.ones(D // 2, F32)])
    ctk = np.ascontiguousarray(cos[pos].T)                      # [64, T]
    stk = np.ascontiguousarray((sin[pos] * sign).T)             # [64, T]
    ctk2 = np.ascontiguousarray(np.stack([ctk, stk], 1))        # [64, 2, T]
    scale = F32(1.0 / np.sqrt(D))
    ctq2 = np.ascontiguousarray(
        np.stack([np.concatenate([ctk, ctk], 0) * scale,
                  np.concatenate([stk, stk], 0) * scale], 1))   # [128, 2, T]

    # single 128x128 lower-tri mask, doubled for the two heads of a pair
    kl = np.arange(128)
    msq = (kl[None, :] >= kl[:, None]).astype(BF16)
    mq = np.ascontiguousarray(np.stack([msq, msq], 1))          # [128, 2, 128]

    in_maps = []
    for c in range(NC):
        wq_cc = np.ascontiguousarray(
            wq[:, c * LH * D:(c + 1) * LH * D].astype(BF16)
            .reshape(16, 128, LH * D).transpose(1, 0, 2))       # [128,16,256]
        wkv_cc = np.ascontiguousarray(
            np.concatenate([wk[:, c * D:(c + 1) * D],
                            wv[:, c * D:(c + 1) * D]], 1).astype(BF16)
            .reshape(16, 128, 2 * D).transpose(1, 0, 2))        # [128,16,128]
        in_maps.append({
            "xT": xprep, "wq_c": wq_cc, "wkv_c": wkv_cc, "wo": woprep,
            "ctq": ctq2, "ctk": ctk2, "mq": mq,
        })
    return in_maps


def get_nc():
    if "nc" not in _CACHE:
        _CACHE["nc"] = _build()
    return _CACHE["nc"]


def run(in_maps, **kwargs):
    nc = get_nc()
    return run_bass_kernel_spmd(nc, in_maps, core_ids=list(range(NC)), **kwargs)


def kernel(x, cos, sin, wq, wk, wv, wo):
    in_maps = _prep_inputs(x, cos, sin, wq, wk, wv, wo)
    res = run(in_maps)
    out = np.empty((T, HID), F32)
    for c in range(NC):
        out[TL * c:TL * (c + 1)] = res.results[c]["out"]
    return out.reshape(B, S, HID)
